# revision 36
# baseline (speedup 1.0000x reference)
"""Trainium2 Bass kernel for a single-head causal attention block.

Reference computation (B=4, T=2048, C=1024, H=64):
    q = x @ Wq; k = x @ Wk; v = x @ Wv          # [B,T,H]
    scores = (q @ k^T) * C**-0.5                # causal masked
    out = softmax(scores) @ v                   # [B,T,H]

Sharding: 2 cores per batch (8 cores, B=4). Core (b, t) owns the 4
interleaved 256-row query chunks {t, t+2, t+4, t+6} of batch b, which
balances causal work exactly across the pair. One uniform SPMD program;
all per-core differences are input data (row arrangement + 0/1 masks).

Each core loads only its own 1024 x rows, projects Q/K/V for them, and
swaps K^T/V^T halves with its pair partner via per-tau AllGather
collectives. The resulting kv layout is global-fixed ([even chunks |
odd chunks]), so the single SPMD program is rank-independent; causal
masking is entirely input data. For query-block i (0..3) the program
attends s-units (128 rows each)
  even block: units 0..2i+1  (last pair masked: diag for t=0, ones for t=1)
  odd block:  units 8..9+2i  (last pair masked: zeros for t=0, diag for t=1)

On-chip pipeline per core:
  x^T tiles via PE transposes -> QKV projections (f32r matmuls, [Wk|Wv]
  packed into one 128-col stationary) -> pairwise K/V AllGather ->
  S^T = K.Q^T tiles [s=128,q=256] -> exp on ScalarE (1/32 scale folded
  in; logits are tiny so no max-subtraction) -> causal mask via 4 DVE
  multiplies per q-block -> P^T.[V|1|0] matmul accumulates out^T and the
  softmax denominator together -> PE transpose + reciprocal -> out rows.
"""

import numpy as np

B, T, C, H = 4, 2048, 1024, 64
NCORES = 8
P = 128          # partitions
NCB = C // P     # 8 channel blocks
NT = T // P      # 16 t-chunks in the kv buffer
QB = 256         # query block width
SCALE = float(C) ** -0.5

_CACHE = {}


def _build_program(loop_n=1, fake_cc=False):
    # fake_cc: replace the AllGather with equivalent-volume local DMAs —
    # numerically wrong (peer half duplicated) but timing-equivalent; used
    # only by the For_i benchmark loop, where real collectives desync.
    import contextlib
    import concourse.bacc as bacc
    import concourse.mybir as mybir
    from concourse import tile

    f32 = mybir.dt.float32
    f32r = mybir.dt.float32r
    EXP = mybir.ActivationFunctionType.Exp

    nc = bacc.Bacc("TRN2", target_bir_lowering=False, debug=False,
                   num_devices=NCORES)

    xq_d = nc.dram_tensor("xq", [T // 2, C], f32r, kind="ExternalInput").ap()
    wq_d = nc.dram_tensor("wq", [P, NCB, H], f32r, kind="ExternalInput").ap()
    wkv_d = nc.dram_tensor("wkv", [P, NCB, 2 * H], f32r, kind="ExternalInput").ap()
    iden_d = nc.dram_tensor("iden", [P, P], f32r, kind="ExternalInput").ap()
    mask_d = {m: nc.dram_tensor(f"mask_{m}", [P, QB], f32r,
                                kind="ExternalInput").ap()
              for m in "abcd"}
    y_d = nc.dram_tensor("y", [T // 2, H], f32, kind="ExternalOutput").ap()

    with tile.TileContext(nc) as tc:
        with (
            tc.tile_pool(name="const", bufs=1) as constp,
            tc.tile_pool(name="big", bufs=1) as bigp,
            tc.tile_pool(name="xnat", bufs=3) as xnatp,
            tc.tile_pool(name="exps", bufs=3) as expp,
            tc.tile_pool(name="small", bufs=4) as smallp,
            tc.tile_pool(name="pt", bufs=2, space="PSUM") as psum_t,
            tc.tile_pool(name="psc", bufs=2, space="PSUM") as psum_sc,
            tc.tile_pool(name="po", bufs=2, space="PSUM") as psum_o,
            tc.tile_pool(name="dram", bufs=1, space="DRAM") as dramp,
        ):
          with (tc.For_i(0, loop_n, 1,
                         hint_engines=(mybir.EngineType.PE,
                                       mybir.EngineType.SP,
                                       mybir.EngineType.Activation,
                                       mybir.EngineType.DVE,
                                       mybir.EngineType.Pool))
                if loop_n > 1 else contextlib.nullcontext()):
            iden = constp.tile([P, P], f32r)
            nc.sync.dma_start(iden[:], iden_d)
            wq_s = constp.tile([P, NCB, H], f32r)
            nc.gpsimd.dma_start(wq_s[:], wq_d)
            wkv_s = constp.tile([P, NCB, 2 * H], f32r)
            nc.gpsimd.dma_start(wkv_s[:], wkv_d)
            mask_s = {}
            for m in "abcd":
                mask_s[m] = constp.tile([P, QB], f32r, name=f"mask_{m}_s",
                                        tag=f"mask_{m}_s")
                nc.gpsimd.dma_start(mask_s[m][:], mask_d[m])
            zbias = constp.tile([P, 1], f32)
            nc.vector.memset(zbias[:], 0.0)
            # warm the ACT exp table-set early (one-time ~2.7us DMA load
            # otherwise lands on the attention critical path)
            expwarm = constp.tile([P, 1], f32)
            nc.scalar.activation(expwarm[:], zbias[:], EXP, bias=zbias[:])

            # ---- x^T per tau-block: 2 tiles [c_in_block, cb, 512] ----
            xT = [bigp.tile([P, NCB, 512], f32r, name=f"xT{tau}",
                            tag=f"xT{tau}") for tau in range(2)]
            kvT_own = [bigp.tile([P, 512], f32r, name=f"kvTo{tau}",
                                 tag=f"kvTo{tau}") for tau in range(2)]
            in_cc = [dramp.tile([P, 512], f32r, name=f"incc{tau}",
                                tag=f"incc{tau}") for tau in range(2)]
            out_cc = [dramp.tile([2 * P, 512], f32r, name=f"outcc{tau}",
                                 tag=f"outcc{tau}") for tau in range(2)]
            kvT = [bigp.tile([P, 512], f32r, name=f"kvT{tau}",
                             tag=f"kvT{tau}") for tau in range(4)]
            for ti in range(NT // 2):
                xh = [xnatp.tile([P, C // 2], f32r, name=f"xn{h}",
                                 tag=f"xn{h}") for h in range(2)]
                for h in range(2):
                    for q4 in range(2):
                        col = h * 512 + q4 * 256
                        nc.sync.dma_start(
                            xh[h][:, q4 * 256:(q4 + 1) * 256],
                            xq_d[ti * P:(ti + 1) * P, col:col + 256])
                for g in range(2):
                    pt = psum_t.tile([P, 512], f32r, tag="pt")
                    for k in range(4):
                        cb = g * 4 + k
                        nc.tensor.transpose(
                            pt[:, k * P:(k + 1) * P],
                            xh[g][:, (cb % 4) * P:(cb % 4 + 1) * P], iden[:])
                    dst = xT[ti // 4][:, g * 4:(g + 1) * 4,
                                      (ti % 4) * P:(ti % 4 + 1) * P]
                    src = pt[:].rearrange("p (g k) -> p g k", g=4)
                    if (ti * 2 + g) % 2 == 0:
                        nc.scalar.copy(dst, src)
                    else:
                        nc.vector.tensor_copy(dst, src)
                if ti % 4 == 3:
                    # this tau's x^T is complete: project K,V and ship half
                    tau = ti // 4
                    pkv = psum_t.tile([P, 512], f32, tag="pt")
                    for cb in range(NCB):
                        nc.tensor.matmul(
                            pkv[:], wkv_s[:, cb, :],
                            xT[tau][:, cb, :],
                            start=(cb == 0), stop=(cb == NCB - 1))
                    if tau % 2 == 0:
                        nc.scalar.copy(kvT_own[tau][:], pkv[:])
                    else:
                        nc.vector.tensor_copy(kvT_own[tau][:], pkv[:])
                    nc.gpsimd.dma_start(in_cc[tau][:], kvT_own[tau][:])
                    if fake_cc:
                        nc.gpsimd.dma_start(out_cc[tau][0:P, :], in_cc[tau][:])
                        nc.gpsimd.dma_start(out_cc[tau][P:2 * P, :],
                                            in_cc[tau][:])
                    else:
                        nc.gpsimd.collective_compute(
                            "AllGather",
                            mybir.AluOpType.bypass,
                            replica_groups=[[2 * b, 2 * b + 1]
                                            for b in range(NCORES // 2)],
                            ins=[in_cc[tau].opt()],
                            outs=[out_cc[tau].opt()],
                        )
                    # kvT[tau] = even-chunk block, kvT[2+tau] = odd-chunk block
                    nc.sync.dma_start(kvT[tau][:], out_cc[tau][0:P, :])
                    nc.sync.dma_start(kvT[2 + tau][:], out_cc[tau][P:2 * P, :])



            qT = [bigp.tile([H, 512], f32r, name=f"qT{tau}",
                            tag=f"qT{tau}") for tau in range(2)]
            for tau in range(2):
                pq = psum_t.tile([H, 512], f32, tag="pt")
                for cb in range(NCB):
                    nc.tensor.matmul(
                        pq[:], wq_s[:, cb, :],
                        xT[tau][:, cb, :],
                        start=(cb == 0), stop=(cb == NCB - 1))
                if tau % 2 == 0:
                    nc.scalar.copy(qT[tau][:], pq[:])
                else:
                    nc.vector.tensor_copy(qT[tau][:], pq[:])

            # ---- V' = [V | 1 | 0] per s-unit: [128, u, 66] ----
            # (66 columns: fp32r matmul dst patterns must be even-sized)
            ones16 = constp.tile([P, NT], f32)
            nc.vector.memset(ones16[:], 1.0)
            zeros16 = constp.tile([P, NT], f32)
            nc.vector.memset(zeros16[:], 0.0)
            vp = [bigp.tile([P, 4, H + 2], f32r, name=f"vp{tau}",
                            tag=f"vp{tau}") for tau in range(4)]
            for tau in range(4):
                nc.vector.tensor_copy(
                    vp[tau][:, :, H:H + 1],
                    ones16[:, 0:4].rearrange("p (u o) -> p u o", o=1))
                nc.vector.tensor_copy(
                    vp[tau][:, :, H + 1:H + 2],
                    zeros16[:, 0:4].rearrange("p (u o) -> p u o", o=1))
            # collective-0-dependent units first (kvT[0], kvT[2])
            for u in [0, 1, 2, 3, 8, 9, 10, 11, 4, 5, 6, 7, 12, 13, 14, 15]:
                pv = psum_t.tile([P, H], f32r, tag="pt")
                nc.tensor.transpose(
                    pv[:], kvT[u // 4][H:2 * H, (u % 4) * P:(u % 4 + 1) * P],
                    iden[H:2 * H, H:2 * H])
                nc.vector.tensor_copy(vp[u // 4][:, u % 4, 0:H], pv[:])

            # ---- attention: software-pipelined across (qb, group) ----
            # The PE runs in program order, so scores for stage p+1 are
            # emitted BEFORE the PV matmuls of stage p; the exp/mask chain
            # of stage p then overlaps the next scores group instead of
            # stalling the PE.
            pairs = []
            for i in range(4):
                seq = (list(range(0, 2 * i + 2))
                       + list(range(8, 8 + 2 * i + 2)))
                for g in range(i + 1):
                    pairs.append((i, g, i + 1, seq))

            po_t = {}
            es_t = {}

            def emit_scores(p):
                i, g, G, seq = pairs[p]
                ps = psum_sc.tile([P, 4 * QB], f32, tag="ps", name=f"ps{p}")
                for k in range(4):
                    u = seq[4 * g + k]
                    nc.tensor.matmul(
                        ps[:, k * QB:(k + 1) * QB],
                        kvT[u // 4][0:H, (u % 4) * P:(u % 4 + 1) * P],
                        qT[i // 2][0:H, (i % 2) * QB:(i % 2 + 1) * QB],
                        start=True, stop=True)
                es = expp.tile([P, 4 * QB], f32r, tag="es", name=f"es{p}")
                for eh in range(2):
                    sl = slice(eh * 2 * QB, (eh + 1) * 2 * QB)
                    nc.scalar.activation(es[:, sl], ps[:, sl], EXP,
                                         bias=zbias[:], scale=SCALE)
                for k in range(4):
                    pos = 4 * g + k
                    m = {2 * i: "a", 2 * i + 1: "b",
                         4 * i + 2: "c", 4 * i + 3: "d"}.get(pos)
                    if m is not None:
                        sl = es[:, k * QB:(k + 1) * QB]
                        nc.vector.tensor_mul(sl, sl, mask_s[m][:])
                es_t[p] = es

            def emit_pv(p):
                i, g, G, seq = pairs[p]
                if g == 0:
                    po_t[i] = psum_o.tile([H + 2, QB], f32, tag="po",
                                          name=f"po{i}")
                es = es_t.pop(p)
                for k in range(4):
                    u = seq[4 * g + k]
                    nc.tensor.matmul(
                        po_t[i][:], vp[u // 4][:, u % 4, 0:H + 2],
                        es[:, k * QB:(k + 1) * QB],
                        start=(g == 0 and k == 0),
                        stop=(g == G - 1 and k == 3))
                if g == G - 1:
                    emit_out(i, po_t.pop(i))

            def emit_out(i, po):
                ot = smallp.tile([H + 2, QB], f32r, tag="ot", name=f"ot{i}")
                nc.vector.tensor_copy(ot[:], po[:])
                for h2 in range(2):
                    pt2 = psum_t.tile([P, H + 2], f32r, tag="pt",
                                      name=f"pt2_{i}_{h2}")
                    nc.tensor.transpose(
                        pt2[:], ot[0:H + 2, h2 * P:(h2 + 1) * P],
                        iden[0:H + 2, 0:H + 2])
                    rc = smallp.tile([P, 1], f32, tag="rc",
                                     name=f"rc{i}{h2}")
                    nc.vector.reciprocal(rc[:], pt2[:, H:H + 1])
                    ys = smallp.tile([P, H], f32, tag="ys",
                                     name=f"ys{i}{h2}")
                    nc.vector.tensor_scalar_mul(ys[:], pt2[:, 0:H], rc[:])
                    nc.sync.dma_start(
                        y_d[i * QB + h2 * P: i * QB + (h2 + 1) * P, :],
                        ys[:])

            for p in range(len(pairs)):
                emit_scores(p)
                if p >= 1:
                    emit_pv(p - 1)
            emit_pv(len(pairs) - 1)

    nc.compile()
    return nc


def _make_masks():
    i = np.arange(P)[:, None]
    j = np.arange(QB)[None, :]
    ma = (i <= j).astype(np.float32)
    mb = (i + P <= j).astype(np.float32)
    return ma, mb


def make_in_maps(x, Wq, Wk, Wv):
    """Per-core input dicts. Core 2*b + t owns query chunks {t, t+2, t+4, t+6}.

    kv layout after the pairwise AllGather is global-fixed:
    s-unit pair {2k, 2k+1} = global chunk 2k (even), pair {8+2k, 9+2k} =
    global chunk 2k+1 (odd). For query-block i (global chunk g = 2i+t) the
    program masks the last pair of each block:
      t=0: even pair i is the diagonal (Ma/Mb), odd pair i is acausal (0);
      t=1: even pair i is fully valid (1), odd pair i is the diagonal.
    """
    x = np.ascontiguousarray(x, dtype=np.float32)
    wkv = np.concatenate([Wk, Wv], axis=1).astype(np.float32)
    wkv = np.ascontiguousarray(wkv.reshape(NCB, P, 2 * H).transpose(1, 0, 2))
    wq = np.ascontiguousarray(
        np.asarray(Wq, np.float32).reshape(NCB, P, H).transpose(1, 0, 2))
    iden = np.eye(P, dtype=np.float32)
    ma, mb = _make_masks()
    ones = np.ones((P, QB), np.float32)
    zeros = np.zeros((P, QB), np.float32)
    xc = x.reshape(B, 8, QB, C)
    in_maps = []
    for core in range(NCORES):
        b, t = divmod(core, 2)
        own = [2 * k + t for k in range(4)]
        xq = np.ascontiguousarray(xc[b, own].reshape(T // 2, C))
        if t == 0:
            m1a, m1b, m2a, m2b = ma, mb, zeros, zeros
        else:
            m1a, m1b, m2a, m2b = ones, ones, ma, mb
        in_maps.append({
            "xq": xq, "wq": wq, "wkv": wkv, "iden": iden,
            "mask_a": m1a, "mask_b": m1b, "mask_c": m2a, "mask_d": m2b,
        })
    return in_maps


def assemble(results):
    y = np.empty((B, T, H), np.float32)
    for core in range(NCORES):
        b, t = divmod(core, 2)
        yc = results[core]["y"]
        for i in range(4):
            g = 2 * i + t
            y[b, g * QB:(g + 1) * QB, :] = yc[i * QB:(i + 1) * QB, :]
    return y


def kernel(x, Wq, Wk, Wv):
    from concourse.bass_utils import run_bass_kernel_spmd
    if "nc" not in _CACHE:
        _CACHE["nc"] = _build_program()
    nc = _CACHE["nc"]
    in_maps = make_in_maps(x, Wq, Wk, Wv)
    res = run_bass_kernel_spmd(nc, in_maps, list(range(NCORES)))
    return assemble(res.results)
